# revision 2
# baseline (speedup 1.0000x reference)
"""Trainium2 Bass kernel for nn_AggregateJoint (grouped 2-layer MLP over parts).

Math: for each of R = b*f rows (x transposed to [R, n=64]), 16 parts each take
4 contiguous channels -> Linear(4,16) -> LeakyReLU -> Linear(16,3) -> BatchNorm
(running stats) -> LeakyReLU -> concat to 48 channels -> output [b, 48, f].

Mapping: per batch b_idx, X = x[b_idx] is [64, 512] with rows=channels and
columns=the 512 f-positions. Both layers are block-diagonal matmuls:
  stage 1: h = leaky(A1 @ X + b1), A1 block-diag [256, 64] (16 blocks 16x4)
  stage 2: y = leaky(A2 @ h + c2), A2 block-diag [48, 256] (BN scale folded)

Per NeuronCore (8-way batch-parallel, 32 batches each), per pair of batches:
  - one 256KB DMA loads x[2t:2t+2] as SBUF [128, 512] (two 64-channel stacks)
  - stage 1: 4 concurrent row-tiled fp32r matmuls (K=32 each, tile_position
    (32i, 0)) -> two PSUM tiles [128, 1024] (parts 0-7 / parts 8-15, the two
    batches side by side in the free dim)
  - epilogue 1: leaky+bias PSUM->SBUF, one ACT op (Lrelu) and one custom DVE
    op per pair (split across both engines; they are the bottleneck)
  - stage 2: 4 accumulating fp32r matmuls (M=48, dst partition base 0) ->
    PSUM [48, 1024]
  - epilogue 2: leaky+bias -> SBUF [48, 1024], alternating engine
  - one 192KB DMA stores both batches' [48, 512] outputs
"""
import os
import sys
import types

import numpy as np

P, IN, H, O = 16, 4, 16, 3
NEG = 0.01
BN_EPS = 1e-5
NCORES = 8
B, N, F = 256, 64, 512
BPC = B // NCORES          # batches per core
PAIRS = BPC // 2

_cache = {}


def _install_ntff_hook():
    """antenv.axon_hooks is absent in this image; recreate it and register the
    ctypes NTFF hook so trace=True works (used by test.py, harmless otherwise)."""
    import antenv

    if "antenv.axon_hooks" in sys.modules:
        return
    m = types.ModuleType("antenv.axon_hooks")
    m._hook = None
    m.set_axon_ntff_profile_hook = lambda h: setattr(m, "_hook", h)
    m.get_axon_ntff_profile_hook = lambda: m._hook
    sys.modules["antenv.axon_hooks"] = m
    antenv.axon_hooks = m
    try:
        from trn_agent_boot.trn_boot import _ntff_profile_via_ctypes

        m._hook = _ntff_profile_via_ctypes("/opt/axon/libaxon_pjrt.so")
    except Exception:
        pass


def _register_leaky():
    """Custom DVE op: out = relu(in0 + s0)*s1 + (in0 + s0)*imm2.
    With s1=0.99, imm2=0.01 this is leaky_relu(in0 + bias) in one pass."""
    import concourse.dve_ops as dve_ops
    from concourse.dve_spec import Spec, Src0, C0, C1, C2, relu, lower
    from concourse.dve_uop import DveOpSpec

    name = "LEAKY_BIAS_ANT"
    if name in dve_ops._SUB_OPCODE_FOR_NAME:
        return next(op for op in dve_ops.OPS if op.name == name)

    def ref(in0, in1, s0, s1, imm2):
        z = in0.astype(np.float32) + s0
        zc = np.nan_to_num(z, nan=0.0, posinf=np.inf, neginf=-np.inf)
        return np.maximum(zc, 0) * s1 + z * imm2

    t = Src0 + C0
    spec = Spec(body=relu(t) * C1 + t * C2, reference=ref)
    row = dve_ops._CUSTOM_DVE_ROW_BASE + len(dve_ops.OPS)
    shas = {}
    for ver in ("v3", "v4"):
        uops = lower(spec, ver=ver)
        shas[ver] = DveOpSpec(name=name, opcode=row, uops=uops, rd1_en=False).sha(ver)
    op = dve_ops.DveOp(name, spec, subdim=False, uops_sha=shas)
    dve_ops.OPS.append(op)
    dve_ops.CUSTOM_DVE_SPECS[name] = spec
    dve_ops._SUB_OPCODE_FOR_NAME[name] = row
    return op


def _prep_weights(parts, W1, b1, W2, b2, gamma, beta, mean, var):
    """Host-side packing of the tiny per-part weights into the SBUF layouts."""
    parts = np.asarray(parts)
    assert np.array_equal(parts.ravel(), np.arange(N)), "non-contiguous parts"
    s = (gamma / np.sqrt(var + BN_EPS)).astype(np.float32)          # [P, O]
    c2 = ((b2 - mean) * s + beta).astype(np.float32)                # [P, O]

    # stage-1 lhsT blocks [32, 128]: rows = in-ch local, cols = h-ch (16p+j).
    # SBUF layout: rows 0-31 grp a, 32-63 grp b, 64-95 grp a, 96-127 grp b
    # (one copy per PE row-group so 2 batches run concurrently).
    blk_a = np.zeros((32, 128), np.float32)
    blk_b = np.zeros((32, 128), np.float32)
    for pl in range(8):
        blk_a[4 * pl:4 * pl + 4, 16 * pl:16 * pl + 16] = W1[pl]
        blk_b[4 * pl:4 * pl + 4, 16 * pl:16 * pl + 16] = W1[8 + pl]
    w1sb = np.concatenate([blk_a, blk_b, blk_a, blk_b], axis=0)     # [128, 128]

    # stage-2 zero-padded lhsT slabs [128, 48] each (accumulating pair)
    w2s = (W2 * s[:, None, :]).astype(np.float32)                   # [P, H, O]
    pad_a = np.zeros((128, 48), np.float32)
    pad_b = np.zeros((128, 48), np.float32)
    for pl in range(8):
        pad_a[16 * pl:16 * pl + 16, 3 * pl:3 * pl + 3] = w2s[pl]
        pad_b[16 * pl:16 * pl + 16, 24 + 3 * pl:24 + 3 * pl + 3] = w2s[8 + pl]
    w2sb = np.concatenate([pad_a, pad_b], axis=1)                   # [128, 96]

    b1a = np.zeros((128, 1), np.float32)
    b1b = np.zeros((128, 1), np.float32)
    for pl in range(8):
        b1a[16 * pl:16 * pl + 16, 0] = b1[pl]
        b1b[16 * pl:16 * pl + 16, 0] = b1[8 + pl]
    b2v = c2.reshape(48, 1)                                         # [48, 1]
    return w1sb, w2sb, b1a, b1b, b2v


def _build():
    _install_ntff_hook()
    LEAKY = _register_leaky()

    from contextlib import ExitStack

    import concourse.bacc as bacc
    import concourse.tile as tile
    import concourse.mybir as mybir

    f32 = mybir.dt.float32
    f32r = mybir.dt.float32r
    AF = mybir.ActivationFunctionType

    nc = bacc.Bacc("TRN2", target_bir_lowering=False, debug=False)

    x_d = nc.dram_tensor("x", [BPC, N, F], f32r, kind="ExternalInput").ap()
    w1_d = nc.dram_tensor("w1", [128, 128], f32r, kind="ExternalInput").ap()
    w2_d = nc.dram_tensor("w2", [128, 96], f32r, kind="ExternalInput").ap()
    b1a_d = nc.dram_tensor("b1a", [128, 1], f32, kind="ExternalInput").ap()
    b1b_d = nc.dram_tensor("b1b", [128, 1], f32, kind="ExternalInput").ap()
    b2_d = nc.dram_tensor("b2", [48, 1], f32, kind="ExternalInput").ap()
    y_d = nc.dram_tensor("y", [BPC, 48, F], f32, kind="ExternalOutput").ap()

    x_pairs = x_d.rearrange("(t q) n f -> t (q n) f", q=2)     # [16, 128, 512]
    y_pairs = y_d.rearrange("(t q) c f -> t c q f", q=2)       # [16, 48, 2, 512]

    with tile.TileContext(nc) as tc, ExitStack() as ctx:
        singles = ctx.enter_context(tc.tile_pool(name="singles", bufs=1))
        xp = ctx.enter_context(tc.tile_pool(name="xp", bufs=4))
        hsb = ctx.enter_context(tc.tile_pool(name="hsb", bufs=3))
        osb = ctx.enter_context(tc.tile_pool(name="osb", bufs=3))
        hps = ctx.enter_context(tc.tile_pool(name="hps", bufs=3, space="PSUM"))
        ops = ctx.enter_context(tc.tile_pool(name="ops", bufs=1, space="PSUM"))

        w1_sb = singles.tile([128, 128], f32r)
        w2_sb = singles.tile([128, 96], f32r)
        b1a_sb = singles.tile([128, 1], f32)
        b1b_sb = singles.tile([128, 1], f32)
        b2_sb = singles.tile([48, 1], f32)
        nc.sync.dma_start(out=w1_sb, in_=w1_d)
        nc.sync.dma_start(out=w2_sb, in_=w2_d)
        nc.sync.dma_start(out=b1a_sb, in_=b1a_d)
        nc.sync.dma_start(out=b1b_sb, in_=b1b_d)
        nc.sync.dma_start(out=b2_sb, in_=b2_d)

        for t in range(PAIRS):
            x_sb = xp.tile([128, F], f32r, tag="x")
            nc.sync.dma_start(out=x_sb, in_=x_pairs[t])

            ps_a = hps.tile([128, 1024], f32, tag="h")
            ps_b = hps.tile([128, 1024], f32, tag="h")
            nc.tensor.matmul(ps_a[:, 0:512], w1_sb[0:32, :], x_sb[0:32, :],
                             start=True, stop=True, tile_position=(0, 0))
            nc.tensor.matmul(ps_b[:, 0:512], w1_sb[32:64, :], x_sb[32:64, :],
                             start=True, stop=True, tile_position=(32, 0))
            nc.tensor.matmul(ps_a[:, 512:1024], w1_sb[64:96, :], x_sb[64:96, :],
                             start=True, stop=True, tile_position=(64, 0))
            nc.tensor.matmul(ps_b[:, 512:1024], w1_sb[96:128, :], x_sb[96:128, :],
                             start=True, stop=True, tile_position=(96, 0))

            h_a = hsb.tile([128, 1024], f32r, tag="ha")
            h_b = hsb.tile([128, 1024], f32r, tag="hb")
            # split the two epilogue-1 ops across ACT and DVE each pair
            nc.scalar.activation(h_a, ps_a, AF.Lrelu, bias=b1a_sb[:, :],
                                 scale=1.0, alpha=NEG)
            nc.vector._custom_dve(LEAKY, out=h_b, in0=ps_b, s0=b1b_sb[:, :],
                                  s1=1.0 - NEG, imm2=NEG)

            ps_o = ops.tile([48, 1024], f32, tag="o")
            nc.tensor.matmul(ps_o[:, 0:512], w2_sb[:, 0:48], h_a[:, 0:512],
                             start=True, stop=False)
            nc.tensor.matmul(ps_o[:, 0:512], w2_sb[:, 48:96], h_b[:, 0:512],
                             start=False, stop=True)
            nc.tensor.matmul(ps_o[:, 512:1024], w2_sb[:, 0:48], h_a[:, 512:1024],
                             start=True, stop=False)
            nc.tensor.matmul(ps_o[:, 512:1024], w2_sb[:, 48:96], h_b[:, 512:1024],
                             start=False, stop=True)

            o_sb = osb.tile([48, 1024], f32, tag="out")
            if t % 2 == 0:
                nc.scalar.activation(o_sb, ps_o, AF.Lrelu, bias=b2_sb[:, :],
                                     scale=1.0, alpha=NEG)
            else:
                nc.vector._custom_dve(LEAKY, out=o_sb, in0=ps_o, s0=b2_sb[:, :],
                                      s1=1.0 - NEG, imm2=NEG)

            nc.sync.dma_start(out=y_pairs[t],
                              in_=o_sb.rearrange("c (q f) -> c q f", q=2))

    nc.compile()
    return nc


def kernel(**inputs):
    import concourse.bass_utils as bass_utils

    if "nc" not in _cache:
        _cache["nc"] = _build()
        bass_utils.upload_artifacts = lambda tmpdir: "local://" + tmpdir
    nc = _cache["nc"]

    x = np.ascontiguousarray(np.asarray(inputs["x"], dtype=np.float32))
    w1sb, w2sb, b1a, b1b, b2v = _prep_weights(
        inputs["parts"],
        np.asarray(inputs["W1"], np.float32), np.asarray(inputs["b1"], np.float32),
        np.asarray(inputs["W2"], np.float32), np.asarray(inputs["b2"], np.float32),
        np.asarray(inputs["gamma"], np.float32), np.asarray(inputs["beta"], np.float32),
        np.asarray(inputs["mean"], np.float32), np.asarray(inputs["var"], np.float32),
    )

    in_maps = []
    for c in range(NCORES):
        in_maps.append({
            "x": x[c * BPC:(c + 1) * BPC],
            "w1": w1sb, "w2": w2sb, "b1a": b1a, "b1b": b1b, "b2": b2v,
        })

    trace = bool(os.environ.get("KERNEL_TRACE"))
    kw = {}
    if trace:
        kw = dict(trace=True, trace_cores=[0], tmpdir=os.environ.get("KERNEL_TRACE_DIR"))
    res = bass_utils.run_bass_kernel_spmd(
        nc, in_maps, core_ids=list(range(NCORES)), **kw)
    _cache["last_result"] = res

    out = np.concatenate([r["y"] for r in res.results], axis=0)  # [256, 48, 512]
    return out


# revision 12
# speedup vs baseline: 1.2225x; 1.2225x over previous
"""Trainium2 Bass kernel for nn_AggregateJoint (grouped 2-layer MLP over parts).

Math: for each of R = b*f rows (x transposed to [R, n=64]), 16 parts each take
4 contiguous channels -> Linear(4,16) -> LeakyReLU -> Linear(16,3) -> BatchNorm
(running stats) -> LeakyReLU -> concat to 48 channels -> output [b, 48, f].

Mapping: per batch b_idx, X = x[b_idx] is [64, 512] (rows=channels, cols=f).
Both layers are block-diagonal matmuls over columns:
  stage 1: h = leaky(A1 @ X + b1), A1 block-diag [256, 64] (16 blocks 16x4)
  stage 2: y = leaky(A2 @ h + c2), A2 block-diag [48, 256] (BN scale folded)

Per NeuronCore (8-way batch-parallel, 32 batches each), per pair of batches:
  - stage 1: 4 concurrent row-tiled fp32r matmuls (K=32, tile_position (32i,0))
    -> two PSUM tiles [128, 1024] (part-group a / b, batches side by side)
  - epilogue 1: leaky+bias PSUM->SBUF bf16 h, one ACT (Lrelu) + one custom DVE
    op per pair (these engines are the bottleneck; bf16 h enables stage-2 col
    tiling)
  - stage 2: 4 concurrent col-tiled bf16 matmuls (M=24, tile_position (0,32j))
    -> PSUM [128, 512] rows 0-23/32-55/64-87/96-119
  - epilogue 2: leaky+bias -> SBUF [120, 512] fp32, alternating engine
  - input DMA: 512KB per two pairs on the sync ring; output DMA: 96KB per pair
    on the gpsimd ring (keeps the sync ring free for input streaming)
"""
import os
import sys
import types

import numpy as np

P, IN, H, O = 16, 4, 16, 3
NEG = 0.01
BN_EPS = 1e-5
NCORES = 8
B, N, F = 256, 64, 512
BPC = B // NCORES          # batches per core
PAIRS = BPC // 2

_cache = {}


def _install_ntff_hook():
    """antenv.axon_hooks is absent in this image; recreate it and register the
    ctypes NTFF hook so trace=True works (used by test.py, harmless otherwise)."""
    import antenv

    if "antenv.axon_hooks" in sys.modules:
        return
    m = types.ModuleType("antenv.axon_hooks")
    m._hook = None
    m.set_axon_ntff_profile_hook = lambda h: setattr(m, "_hook", h)
    m.get_axon_ntff_profile_hook = lambda: m._hook
    sys.modules["antenv.axon_hooks"] = m
    antenv.axon_hooks = m
    try:
        from trn_agent_boot.trn_boot import _ntff_profile_via_ctypes

        m._hook = _ntff_profile_via_ctypes("/opt/axon/libaxon_pjrt.so")
    except Exception:
        pass


def _register_leaky():
    """Custom DVE op: out = relu(in0 + s0)*s1 + (in0 + s0)*imm2.
    With s1=0.99, imm2=0.01 this is leaky_relu(in0 + bias) in one pass."""
    import concourse.dve_ops as dve_ops
    from concourse.dve_spec import Spec, Src0, C0, C1, C2, relu, lower
    from concourse.dve_uop import DveOpSpec

    name = "LEAKY_BIAS_ANT"
    if name in dve_ops._SUB_OPCODE_FOR_NAME:
        return next(op for op in dve_ops.OPS if op.name == name)

    def ref(in0, in1, s0, s1, imm2):
        z = in0.astype(np.float32) + s0
        zc = np.nan_to_num(z, nan=0.0, posinf=np.inf, neginf=-np.inf)
        return np.maximum(zc, 0) * s1 + z * imm2

    t = Src0 + C0
    spec = Spec(body=relu(t) * C1 + t * C2, reference=ref)
    row = dve_ops._CUSTOM_DVE_ROW_BASE + len(dve_ops.OPS)
    shas = {}
    for ver in ("v3", "v4"):
        uops = lower(spec, ver=ver)
        shas[ver] = DveOpSpec(name=name, opcode=row, uops=uops, rd1_en=False).sha(ver)
    op = dve_ops.DveOp(name, spec, subdim=False, uops_sha=shas)
    dve_ops.OPS.append(op)
    dve_ops.CUSTOM_DVE_SPECS[name] = spec
    dve_ops._SUB_OPCODE_FOR_NAME[name] = row
    return op


def _prep_weights(parts, W1, b1, W2, b2, gamma, beta, mean, var):
    """Host-side packing of the tiny per-part weights into SBUF layouts."""
    parts = np.asarray(parts)
    assert np.array_equal(parts.ravel(), np.arange(N)), "non-contiguous parts"
    s = (gamma / np.sqrt(var + BN_EPS)).astype(np.float32)          # [P, O]
    c2 = ((b2 - mean) * s + beta).astype(np.float32)                # [P, O]

    # stage-1 lhsT blocks [32, 128]: rows = in-ch local, cols = h-ch (16p+j).
    # SBUF rows: 0-31 grp a, 32-63 grp b, 64-95 grp a, 96-127 grp b (one copy
    # per PE row-group so two batches run concurrently).
    blk_a = np.zeros((32, 128), np.float32)
    blk_b = np.zeros((32, 128), np.float32)
    for pl in range(8):
        blk_a[4 * pl:4 * pl + 4, 16 * pl:16 * pl + 16] = W1[pl]
        blk_b[4 * pl:4 * pl + 4, 16 * pl:16 * pl + 16] = W1[8 + pl]
    w1sb = np.concatenate([blk_a, blk_b, blk_a, blk_b], axis=0)     # [128, 128]

    # stage-2 lhsT [128, 48] in bf16: cols 0-23 grp a block, 24-47 grp b block
    w2s = (W2 * s[:, None, :]).astype(np.float32)                   # [P, H, O]
    w2sb = np.zeros((128, 48), np.float32)
    for pl in range(8):
        w2sb[16 * pl:16 * pl + 16, 3 * pl:3 * pl + 3] = w2s[pl]
        w2sb[16 * pl:16 * pl + 16, 24 + 3 * pl:24 + 3 * pl + 3] = w2s[8 + pl]
    import ml_dtypes
    w2sb = w2sb.astype(ml_dtypes.bfloat16)

    b1a = np.zeros((128, 1), np.float32)
    b1b = np.zeros((128, 1), np.float32)
    for pl in range(8):
        b1a[16 * pl:16 * pl + 16, 0] = b1[pl]
        b1b[16 * pl:16 * pl + 16, 0] = b1[8 + pl]
    # ep2 bias on PSUM partition layout: 0-23 c2a, 32-55 c2b, 64-87 c2a,
    # 96-119 c2b (batch-even / batch-odd share values)
    b2v = np.zeros((128, 1), np.float32)
    ca = c2[0:8].reshape(24)
    cb = c2[8:16].reshape(24)
    b2v[0:24, 0] = ca
    b2v[32:56, 0] = cb
    b2v[64:88, 0] = ca
    b2v[96:120, 0] = cb
    return w1sb, w2sb, b1a, b1b, b2v


def _build():
    _install_ntff_hook()
    LEAKY = _register_leaky()

    from contextlib import ExitStack

    import concourse.bacc as bacc
    import concourse.tile as tile
    import concourse.mybir as mybir

    f32 = mybir.dt.float32
    f32r = mybir.dt.float32r
    bf16 = mybir.dt.bfloat16
    AF = mybir.ActivationFunctionType

    nc = bacc.Bacc("TRN2", target_bir_lowering=False, debug=False)

    x_d = nc.dram_tensor("x", [BPC, N, F], f32r, kind="ExternalInput").ap()
    w1_d = nc.dram_tensor("w1", [128, 128], f32r, kind="ExternalInput").ap()
    w2_d = nc.dram_tensor("w2", [128, 48], bf16, kind="ExternalInput").ap()
    b1a_d = nc.dram_tensor("b1a", [128, 1], f32, kind="ExternalInput").ap()
    b1b_d = nc.dram_tensor("b1b", [128, 1], f32, kind="ExternalInput").ap()
    b2_d = nc.dram_tensor("b2", [128, 1], f32, kind="ExternalInput").ap()
    y_d = nc.dram_tensor("y", [BPC, 48, F], f32, kind="ExternalOutput").ap()

    # x as [quads u][pair q][128 rows][512]:  u in 0..7, q in 0..1
    x_q = x_d.rearrange("(u q two) n f -> u (two n) q f", q=2, two=2)  # [8,128,2,512]
    x_pairs = x_d.rearrange("(t two) n f -> t (two n) f", two=2)       # [16,128,512]
    # y as [quad-of-pairs U][parity q][grp g][c 24][pair-in-quad m][f]
    y_quads = y_d.rearrange("(u m q) (g c) f -> u q g c m f", m=4, q=2, g=2)

    with tile.TileContext(nc) as tc, ExitStack() as ctx:
        singles = ctx.enter_context(tc.tile_pool(name="singles", bufs=1))
        xp = ctx.enter_context(tc.tile_pool(name="xp", bufs=3))
        hsb = ctx.enter_context(tc.tile_pool(name="hsb", bufs=3))
        osb = ctx.enter_context(tc.tile_pool(name="osb", bufs=3))
        hps = ctx.enter_context(tc.tile_pool(name="hps", bufs=3, space="PSUM"))
        ops = ctx.enter_context(tc.tile_pool(name="ops", bufs=2, space="PSUM"))

        w1_sb = singles.tile([128, 128], f32r)
        w2_sb = singles.tile([128, 48], bf16)
        b1a_sb = singles.tile([128, 1], f32)
        b1b_sb = singles.tile([128, 1], f32)
        b2_sb = singles.tile([128, 1], f32)
        nc.sync.dma_start(out=w1_sb, in_=w1_d)
        nc.sync.dma_start(out=w2_sb, in_=w2_d)
        nc.sync.dma_start(out=b1a_sb, in_=b1a_d)
        nc.sync.dma_start(out=b1b_sb, in_=b1b_d)
        nc.sync.dma_start(out=b2_sb, in_=b2_d)

        for t in range(PAIRS):
            u, q = divmod(t, 2)
            if q == 0:
                x_sb = xp.tile([128, 2, 512], f32r, tag="x")
                nc.sync.dma_start(out=x_sb, in_=x_q[u])
                _cache.setdefault("xtiles", {})[u] = x_sb
            else:
                x_sb = _cache["xtiles"][u]

            ps_a = hps.tile([128, 1024], f32, tag="h")
            ps_b = hps.tile([128, 1024], f32, tag="h")
            nc.tensor.matmul(ps_a[:, 0:512], w1_sb[0:32, :], x_sb[0:32, q, :],
                             start=True, stop=True, tile_position=(0, 0))
            nc.tensor.matmul(ps_b[:, 0:512], w1_sb[32:64, :], x_sb[32:64, q, :],
                             start=True, stop=True, tile_position=(32, 0))
            nc.tensor.matmul(ps_a[:, 512:1024], w1_sb[64:96, :], x_sb[64:96, q, :],
                             start=True, stop=True, tile_position=(64, 0))
            nc.tensor.matmul(ps_b[:, 512:1024], w1_sb[96:128, :], x_sb[96:128, q, :],
                             start=True, stop=True, tile_position=(96, 0))

            h_a = hsb.tile([128, 1024], bf16, tag="ha")
            h_b = hsb.tile([128, 1024], bf16, tag="hb")
            # split the two epilogue-1 ops across ACT and DVE each pair
            nc.scalar.activation(h_a, ps_a, AF.Lrelu, bias=b1a_sb[:, :],
                                 scale=1.0, alpha=NEG)
            nc.vector._custom_dve(LEAKY, out=h_b, in0=ps_b, s0=b1b_sb[:, :],
                                  s1=1.0 - NEG, imm2=NEG)

            ps_o = ops.tile([128, 512], f32, tag="o")
            nc.tensor.matmul(ps_o[0:24, :], w2_sb[:, 0:24], h_a[:, 0:512],
                             start=True, stop=True, tile_position=(0, 0))
            nc.tensor.matmul(ps_o[32:56, :], w2_sb[:, 24:48], h_b[:, 0:512],
                             start=True, stop=True, tile_position=(0, 32))
            nc.tensor.matmul(ps_o[64:88, :], w2_sb[:, 0:24], h_a[:, 512:1024],
                             start=True, stop=True, tile_position=(0, 64))
            nc.tensor.matmul(ps_o[96:120, :], w2_sb[:, 24:48], h_b[:, 512:1024],
                             start=True, stop=True, tile_position=(0, 96))

            m = t % 4
            if m == 0:
                o_sb = osb.tile([128, 4, 512], f32, tag="out")
                _cache["otile"] = o_sb
            else:
                o_sb = _cache["otile"]
            if t % 2 == 0:
                nc.scalar.activation(o_sb[0:120, m, :], ps_o[0:120, :], AF.Lrelu,
                                     bias=b2_sb[0:120, :], scale=1.0, alpha=NEG)
            else:
                nc.vector._custom_dve(LEAKY, out=o_sb[0:120, m, :], in0=ps_o[0:120, :],
                                      s0=b2_sb[0:120, :], s1=1.0 - NEG, imm2=NEG)

            if m == 3:
                # 4 contiguous-partition block DMAs cover the last 4 pairs
                U = t // 4
                o_blocks = o_sb.rearrange("(gg c) m f -> gg c m f", gg=4)[:, 0:24, :, :]
                for blk in range(4):
                    q, g = divmod(blk, 2)
                    # dst: batches 8U+2m+q (m=0..3), channels 24g..24g+24
                    nc.gpsimd.dma_start(
                        out=y_quads[U, q, g],        # [24, 4, 512]
                        in_=o_blocks[blk])

    _cache.pop("xtiles", None)
    _cache.pop("otile", None)
    nc.compile()
    return nc


def kernel(**inputs):
    import concourse.bass_utils as bass_utils

    if "nc" not in _cache:
        _cache["nc"] = _build()
        bass_utils.upload_artifacts = lambda tmpdir: "local://" + tmpdir
    nc = _cache["nc"]

    x = np.ascontiguousarray(np.asarray(inputs["x"], dtype=np.float32))
    w1sb, w2sb, b1a, b1b, b2v = _prep_weights(
        inputs["parts"],
        np.asarray(inputs["W1"], np.float32), np.asarray(inputs["b1"], np.float32),
        np.asarray(inputs["W2"], np.float32), np.asarray(inputs["b2"], np.float32),
        np.asarray(inputs["gamma"], np.float32), np.asarray(inputs["beta"], np.float32),
        np.asarray(inputs["mean"], np.float32), np.asarray(inputs["var"], np.float32),
    )

    in_maps = []
    for c in range(NCORES):
        in_maps.append({
            "x": x[c * BPC:(c + 1) * BPC],
            "w1": w1sb, "w2": w2sb, "b1a": b1a, "b1b": b1b, "b2": b2v,
        })

    trace = bool(os.environ.get("KERNEL_TRACE"))
    kw = {}
    if trace:
        kw = dict(trace=True, trace_cores=[0], tmpdir=os.environ.get("KERNEL_TRACE_DIR"))
    res = bass_utils.run_bass_kernel_spmd(
        nc, in_maps, core_ids=list(range(NCORES)), **kw)
    _cache["last_result"] = res

    out = np.concatenate([r["y"] for r in res.results], axis=0)  # [256, 48, 512]
    return out
